# revision 1
# baseline (speedup 1.0000x reference)
"""ARMA(4,4) generator as a truncated-impulse-response convolution on TRN2.

Math: the reference recurrence
    x_t = mu + sum_i phi_i x_{t-i} + e_t + sum_j theta_j e_{t-j}
is a linear time-invariant filter applied to u_t = e_t + c_t (where c solves
c_t + sum_j theta_j c_{t-j} = mu, making the mu term exact), independently per
(sequence, channel):
    x[n, t, d] = sum_k g_d[k] * u[n, t-k, d]   (+ response to x0, zero here)
with g_d the ARMA impulse response (g[0] = 1).  g decays geometrically
(stationary filter); each channel's 128-tap block count NK_d is chosen at
runtime from the actual phi/theta so the truncation error stays below 3e-5.

Device kernel: per channel d, the causal convolution over a 128-step time
block is a lower-triangular block-Toeplitz matmul — time on SBUF partitions,
(sequence, block) pairs as matmul columns.  No serial recurrence remains.

Performance notes (measured on TRN2 via neuron-profile):
- matmul rhs columns MUST be contiguous (stride-1): strided columns stream
  at 4 cycles/col, dense at 1 col/cycle.  The host marshals the SBUF image
  d-major so each channel's 512 columns are dense, and every DMA moves
  contiguous per-partition rows at full HBM bandwidth.
- Precision: u and the Toeplitz weights are split into fp16 hi+lo on the
  host; x = Whi@Uhi + Whi@Ulo + Wlo@Uhi accumulates in fp32 PSUM (rel err
  ~2e-7 vs the fp32 reference).  The hi and lo column blocks of one weight
  matmul write the same PSUM columns through a stride-0 output dim, so the
  add happens in-stream and evacuation is a single copy per channel.
- Channels stream through in 4 groups of 16 per chunk-pair so the weight
  working set fits SBUF alongside two 8-sequence input chunks.

Sharding: pure data parallelism — 32 of the 256 sequences per NeuronCore.
"""

import os
import numpy as np

N, T, D, P, Q = 256, 4096, 64, 4, 4
NCORES = 8
SEQ_PER_CORE = N // NCORES          # 32
BLK = 128                           # time block = SBUF partition count
TB = T // BLK                       # 32 time blocks per sequence
CHUNK_SEQS = 4                      # sequences per pipeline chunk
KMAX = 1280                         # host impulse-response horizon
NK_CAP = 8

_CACHE = {}
LAST_EXEC_NS = None


def _impulse_response(phi, theta):
    """g[k, d] in float64 for k = 0..KMAX-1."""
    g = np.zeros((KMAX, D), dtype=np.float64)
    g[0] = 1.0
    phi64 = phi.astype(np.float64)
    th64 = theta.astype(np.float64)
    for k in range(1, KMAX):
        acc = np.zeros(D, dtype=np.float64)
        if k <= Q:
            acc += th64[:, k - 1]
        for i in range(1, P + 1):
            if k - i >= 0:
                acc += phi64[:, i - 1] * g[k - i]
        g[k] = acc
    return g


def _mu_offset(theta, mu):
    """c[t, d] with c_t + sum_j theta_j c_{t-j} = mu for all t >= 0.

    Adding c to eps makes the single ARMA filter g reproduce the mu term
    exactly (mu must not pass through the MA part, so a constant offset
    would be wrong during the first Q steps)."""
    th64 = theta.astype(np.float64)
    mu64 = mu.astype(np.float64)
    c = np.zeros((T, D), dtype=np.float64)
    for t in range(T):
        acc = mu64.copy()
        for j in range(1, Q + 1):
            if t - j >= 0:
                acc -= th64[:, j - 1] * c[t - j]
        c[t] = acc
    if np.abs(c).max() > 100.0 * max(np.abs(mu64).max(), 1.0):
        raise ValueError("MA polynomial near non-invertible; c_t diverges")
    return c


def _pick_nk(g, dc_scale):
    """Per-channel tap-block count: coherent DC bias + 6-sigma random tail."""
    mu64 = np.abs(np.asarray(dc_scale, dtype=np.float64))
    nk_d = np.zeros(D, dtype=int)
    for d in range(D):
        for nk in range(2, NK_CAP + 1):
            # block j covers taps j*BLK + t - t'; at output position t=0 the
            # guaranteed coverage ends at (nk-1)*BLK.
            tail = g[(nk - 1) * BLK + 1 :, d]
            bias = abs(tail.sum()) * mu64[d]
            sigma = np.sqrt((tail**2).sum())
            if bias + 6 * sigma < 3e-5:
                nk_d[d] = nk
                break
        else:
            raise ValueError("impulse response decays too slowly")
    return nk_d


def _toeplitz_pair(g, d, j):
    """W[t', t] = g[j*BLK + t - t', d] (zero where k<0), float64."""
    w = np.zeros((BLK, BLK), dtype=np.float64)
    for tp in range(BLK):
        ks = j * BLK - tp
        lo_t = max(0, -ks)
        w[tp, lo_t:] = g[ks + lo_t : ks + BLK, d]
    return w


def _hi_lo(a):
    hi = a.astype(np.float16)
    lo = (a - hi.astype(np.float64)).astype(np.float16)
    return hi, lo


def _split_waits(nc, limit=1):
    """Walrus in this container rejects instructions carrying more than a
    couple of sync waits.  Move excess waits onto same-engine NOPs placed
    immediately before the offending instruction (program order on the
    engine queue preserves the semantics)."""
    import bass_rust
    import concourse.mybir as mybir

    n_split = 0
    for bb_name, bassbb in list(nc.bb_map.items()):
        bb = bassbb.bb
        insts = list(bb.instructions)
        out = []
        changed = False
        for inst in insts:
            si = inst.sync_info
            if si is not None and len(si.on_wait) > limit:
                waits = list(si.on_wait)
                keep = waits[:limit]
                rest = waits[limit:]
                while rest:
                    chunk, rest = rest[:limit], rest[limit:]
                    nop = bass_rust.InstNoOp(
                        name=f"waitsplit-{n_split}", engine=inst.engine
                    )
                    n_split += 1
                    nop.sync_info = mybir.SyncInfo(on_wait=chunk, on_update=[])
                    nc.register_instruction(nop)
                    out.append(nop)
                inst.sync_info = mybir.SyncInfo(
                    on_wait=keep, on_update=list(si.on_update)
                )
                changed = True
            out.append(inst)
        if changed:
            bb.instructions = out
    return n_split


def _tile_context_cls():
    from concourse.tile import TileContext
    from concourse.vector_clock import ScopedClock, VectorClock

    class TileContextFix(TileContext):
        # This walrus build rejects >2 sync waits on one CTRL instruction
        # ("Too many sync wait commands"), which the stock final drain hits.
        # Split the final-drain waits one-per-NOP on SP; the drain then
        # needs none (program order on SP covers it).
        def _drain_and_barrier(self, tick_clock, wait_clock):
            ticks = list(tick_clock.global_clock)
            for proc, tick in enumerate(ticks):
                if tick <= 0:
                    continue
                nop = self.nc.sync.nop(nofuse=True, hint="drain_wait_split")
                sub = VectorClock(
                    [tick if i == proc else 0 for i in range(len(ticks))]
                )
                wait_clock.add_sem_waits(nop.ins, ScopedClock({None: sub}))
            self.nc.sync.drain()
            self.nc.all_engine_barrier()
            assert self.sems is not None
            popped = self.nc._tile_sem_poison_stack.pop()
            assert popped is self._sem_poison
            self.nc.clear_and_free_semaphores(list(self.sems.allocated().values()))
            self.nc.all_engine_barrier()

    return TileContextFix



_MARSHAL_G = [None]


CS = 8                      # sequences per chunk
NCHUNK = SEQ_PER_CORE // CS  # 4
NG = 4                      # channel groups
DG = D // NG                # 16 channels per group

_CACHE = {}
LAST_EXEC_NS = None


def _group_pairs(nk_key):
    """Per group: list of (d, j) pairs, d within the group."""
    groups = []
    for g in range(NG):
        pairs = []
        for d in range(g * DG, (g + 1) * DG):
            for j in range(nk_key[d]):
                pairs.append((d, j))
        groups.append(pairs)
    return groups


def _build_bass(nk_key):
    import concourse.bass as bass
    import concourse.mybir as mybir

    TileContextFix = _tile_context_cls()
    f16, f32 = mybir.dt.float16, mybir.dt.float32
    nk_d = list(nk_key)
    groups = _group_pairs(nk_key)
    maxp = max(len(p) for p in groups)
    goff = [0]
    for g in range(NG):
        goff.append(goff[-1] + len(groups[g]))
    npairs = goff[-1]

    nc = bass.Bass()
    # input: SBUF-image [chunk][128][CS*TB*2D] fp16, contiguous
    ehl = nc.declare_dram_parameter("ehl", [NCHUNK, BLK, CS * TB * 2 * D], f16, isOutput=False)
    wh_p = nc.declare_dram_parameter("wh", [BLK, npairs * BLK], f16, isOutput=False)
    wl_p = nc.declare_dram_parameter("wl", [BLK, npairs * BLK], f16, isOutput=False)
    # output: SBUF-image [chunk][group][128][DG*CS*TB] f32, contiguous
    xout = nc.declare_dram_parameter("x", [NCHUNK, NG, BLK, DG * CS * TB], f32, isOutput=True)

    NCOL = CS * TB            # 256 (s, tb) columns per channel

    with TileContextFix(nc) as tc:
        with (
            tc.tile_pool(name="wpool", bufs=2) as wpool,
            tc.tile_pool(name="epool", bufs=2) as epool,
            tc.tile_pool(name="opool", bufs=2) as opool,
            tc.tile_pool(name="pspool", bufs=8, space="PSUM") as pspool,
        ):
            for cp in range(NCHUNK // 2):
                ets = []
                for h in range(2):
                    et = epool.tile([BLK, CS * TB * 2 * D], f16, name="et")
                    nc.sync.dma_start(out=et[:], in_=ehl[2 * cp + h])
                    ets.append(et)
                for g in range(NG):
                    gp = groups[g]
                    ngp = len(gp)
                    gp_idx = {p_: i for i, p_ in enumerate(gp)}
                    wht = wpool.tile([BLK, maxp * BLK], f16, name="wht")
                    nc.sync.dma_start(
                        out=wht[:, : ngp * BLK],
                        in_=wh_p[:, goff[g] * BLK : goff[g + 1] * BLK],
                    )
                    wlt = wpool.tile([BLK, maxp * BLK], f16, name="wlt")
                    nc.sync.dma_start(
                        out=wlt[:, : ngp * BLK],
                        in_=wl_p[:, goff[g] * BLK : goff[g + 1] * BLK],
                    )
                    for h in range(2):
                        et = ets[h]
                        ot = opool.tile([BLK, DG * NCOL], f32, name="ot")
                        # et view: [p][d][hl][s][tb] — channel cols contiguous
                        e5 = et.rearrange(
                            "p (d hl s c) -> p d hl s c", d=D, hl=2, s=CS
                        )
                        for dl in range(DG):
                            d = g * DG + dl
                            ps = pspool.tile([BLK, NCOL], f32, name="ps")
                            ps3 = ps.rearrange("p (s c) -> p s c", s=CS)
                            n_mm = 2 * nk_d[d]
                            mi = 0
                            for j in range(nk_d[d]):
                                wi = gp_idx[(d, j)]
                                wh = wht[:, wi * BLK : (wi + 1) * BLK]
                                wl = wlt[:, wi * BLK : (wi + 1) * BLK]
                                # merged hi|lo rhs (hl, s, tb'), tb' contiguous;
                                # both hl halves land on the SAME psum columns
                                # (stride-0 out dim) and accumulate in-stream.
                                rhs_m = e5[:, d, :, :, 0 : TB - j]
                                out_m = (
                                    ps3[:, :, j:]
                                    .unsqueeze(1)
                                    .broadcast_to((BLK, 2, CS, TB - j))
                                )
                                nc.tensor.matmul(
                                    out_m, wh, rhs_m, start=(mi == 0), stop=False
                                )
                                mi += 1
                                rhs_h = e5[:, d, 0, :, 0 : TB - j]
                                out_h = ps3[:, :, j:]
                                nc.tensor.matmul(
                                    out_h, wl, rhs_h,
                                    start=False, stop=(mi == n_mm - 1),
                                )
                                mi += 1
                            dst = ot[:, dl * NCOL : (dl + 1) * NCOL]
                            if dl % 2 == 0:
                                nc.scalar.copy(out=dst, in_=ps[:])
                            else:
                                nc.vector.tensor_copy(out=dst, in_=ps[:])
                        nc.sync.dma_start(out=xout[2 * cp + h, g], in_=ot[:])
    _split_waits(nc)
    return nc, groups


def _marshal_inputs(uhi, ulo, nk_key):
    """Host-side SBUF-image marshaling."""
    u2 = np.empty((N, T, D, 2), dtype=np.float16)
    u2[..., 0] = uhi
    u2[..., 1] = ulo
    # [core, chunk, s, tb, p, d, hl] -> [core, chunk, p, (d, hl, s, tb)]
    a = u2.reshape(NCORES, NCHUNK, CS, TB, BLK, D, 2)
    a = np.ascontiguousarray(a.transpose(0, 1, 4, 5, 6, 2, 3))
    ehl_dev = a.reshape(NCORES, NCHUNK, BLK, CS * TB * 2 * D)

    groups = _group_pairs(nk_key)
    pairs = [p for g in groups for p in g]
    g64 = _MARSHAL_G[0]
    wh = np.empty((BLK, len(pairs), BLK), dtype=np.float16)
    wl = np.empty((BLK, len(pairs), BLK), dtype=np.float16)
    for i, (d, j) in enumerate(pairs):
        w = _toeplitz_pair(g64, d, j)
        wh[:, i, :], wl[:, i, :] = _hi_lo(w)
    return ehl_dev, {
        "wh": np.ascontiguousarray(wh.reshape(BLK, -1)),
        "wl": np.ascontiguousarray(wl.reshape(BLK, -1)),
    }


def _unmarshal_output(res_list):
    x = np.empty((N, T, D), dtype=np.float32)
    for c in range(NCORES):
        arr = res_list[c]["x"]  # [chunk, g, p, DG*CS*TB]
        a = arr.reshape(NCHUNK, NG, BLK, DG, CS, TB)
        # -> [chunk, s, tb, p, g, dl]
        a = a.transpose(0, 4, 5, 2, 1, 3)
        x[c * SEQ_PER_CORE : (c + 1) * SEQ_PER_CORE] = a.reshape(
            SEQ_PER_CORE, T, D
        )
    return x


def kernel(eps, phi, theta, mu, x0):
    global LAST_EXEC_NS
    eps = np.asarray(eps, dtype=np.float32)
    phi = np.asarray(phi, dtype=np.float32)
    theta = np.asarray(theta, dtype=np.float32)
    mu = np.asarray(mu, dtype=np.float32)
    x0 = np.asarray(x0, dtype=np.float32)

    g = _impulse_response(phi, theta)
    _MARSHAL_G[0] = g
    c = _mu_offset(theta, mu)
    nk_d = _pick_nk(g, np.abs(c).max(axis=0))
    nk_key = tuple(int(v) for v in nk_d)

    u = eps.astype(np.float64) + c[None, :, :]
    uhi, ulo = _hi_lo(u)
    ehl_dev, wmaps = _marshal_inputs(uhi, ulo, nk_key)

    if nk_key not in _CACHE:
        _CACHE[nk_key] = _build_bass(nk_key)
    nc, _groups = _CACHE[nk_key]

    from concourse.bass_utils import run_bass_kernel_spmd

    core_ids = list(range(NCORES))
    in_maps = [
        {"ehl": np.ascontiguousarray(ehl_dev[cid]), **wmaps} for cid in core_ids
    ]
    trace = bool(int(os.environ.get("ARMA_TRACE", "0")))
    res = run_bass_kernel_spmd(nc, in_maps, core_ids, trace=trace)
    LAST_EXEC_NS = res.exec_time_ns

    x = _unmarshal_output(res.results)

    if np.any(x0):
        h0 = np.zeros((T, D), dtype=np.float64)
        phi64 = phi.astype(np.float64)
        hist = [np.zeros(D)] * 3 + [np.ones(D)]
        for t in range(T):
            val = (
                phi64[:, 0] * hist[3]
                + phi64[:, 1] * hist[2]
                + phi64[:, 2] * hist[1]
                + phi64[:, 3] * hist[0]
            )
            h0[t] = val
            hist = hist[1:] + [val]
        x = x + (x0[:, None, :] * h0[None, :, :]).astype(np.float32)
    return x



# revision 2
# speedup vs baseline: 2.4670x; 2.4670x over previous
"""ARMA(4,4) generator as a truncated-impulse-response convolution on TRN2.

Math: the reference recurrence
    x_t = mu + sum_i phi_i x_{t-i} + e_t + sum_j theta_j e_{t-j}
is a linear time-invariant filter applied to u_t = e_t + c_t (where c solves
c_t + sum_j theta_j c_{t-j} = mu, making the mu term exact), independently per
(sequence, channel):
    x[n, t, d] = sum_k g_d[k] * u[n, t-k, d]   (+ response to x0, zero here)
with g_d the ARMA impulse response (g[0] = 1).  g decays geometrically
(stationary filter); each channel's 128-tap block count NK_d is chosen at
runtime from the actual phi/theta so the truncation error stays below 3e-5.

Device kernel: per channel d, the causal convolution over a 128-step time
block is a lower-triangular block-Toeplitz matmul — time on SBUF partitions,
(sequence, block) pairs as matmul columns.  No serial recurrence remains.

Performance: the kernel is DMA-bound (cost model: all transfers serialize at
360 GB/s), so everything is fp16 end-to-end — inputs, Toeplitz weights and
the output round-trip HBM at 2 bytes/elem, and the weights are loaded once
(not per chunk).  fp32 PSUM accumulation keeps the conv exact; the only
losses are the three fp16 quantizations (~3e-4 rel err vs the fp32
reference, gate is 2e-2).  Traffic per core: 16.75 MB in + 4 MB weights +
16.75 MB out = 37.5 MB -> ~105 us at the 360 GB/s cost-model bandwidth,
vs 99 MB / ~275 us for the fp32+hi/lo variant.

Sharding: pure data parallelism — 32 of the 256 sequences per NeuronCore.
"""

import os
import numpy as np

N, T, D, P, Q = 256, 4096, 64, 4, 4
NCORES = 8
SEQ_PER_CORE = N // NCORES          # 32
BLK = 128                           # time block = SBUF partition count
TB = T // BLK                       # 32 time blocks per sequence
KMAX = 1280                         # host impulse-response horizon
NK_CAP = 8

CS = 8                      # sequences per chunk
NCHUNK = SEQ_PER_CORE // CS  # 4
NG = 4                      # channel groups
DG = D // NG                # 16 channels per group
NCOL = CS * TB              # 256 (s, tb) columns per channel

_CACHE = {}
LAST_EXEC_NS = None
_MARSHAL_G = [None]


def _impulse_response(phi, theta):
    """g[k, d] in float64 for k = 0..KMAX-1."""
    g = np.zeros((KMAX, D), dtype=np.float64)
    g[0] = 1.0
    phi64 = phi.astype(np.float64)
    th64 = theta.astype(np.float64)
    for k in range(1, KMAX):
        acc = np.zeros(D, dtype=np.float64)
        if k <= Q:
            acc += th64[:, k - 1]
        for i in range(1, P + 1):
            if k - i >= 0:
                acc += phi64[:, i - 1] * g[k - i]
        g[k] = acc
    return g


def _mu_offset(theta, mu):
    """c[t, d] with c_t + sum_j theta_j c_{t-j} = mu for all t >= 0.

    Adding c to eps makes the single ARMA filter g reproduce the mu term
    exactly (mu must not pass through the MA part, so a constant offset
    would be wrong during the first Q steps)."""
    th64 = theta.astype(np.float64)
    mu64 = mu.astype(np.float64)
    c = np.zeros((T, D), dtype=np.float64)
    for t in range(T):
        acc = mu64.copy()
        for j in range(1, Q + 1):
            if t - j >= 0:
                acc -= th64[:, j - 1] * c[t - j]
        c[t] = acc
    if np.abs(c).max() > 100.0 * max(np.abs(mu64).max(), 1.0):
        raise ValueError("MA polynomial near non-invertible; c_t diverges")
    return c


def _pick_nk(g, dc_scale):
    """Per-channel tap-block count: coherent DC bias + 6-sigma random tail."""
    mu64 = np.abs(np.asarray(dc_scale, dtype=np.float64))
    nk_d = np.zeros(D, dtype=int)
    for d in range(D):
        for nk in range(2, NK_CAP + 1):
            # block j covers taps j*BLK + t - t'; at output position t=0 the
            # guaranteed coverage ends at (nk-1)*BLK.
            tail = g[(nk - 1) * BLK + 1 :, d]
            bias = abs(tail.sum()) * mu64[d]
            sigma = np.sqrt((tail**2).sum())
            if bias + 6 * sigma < 3e-5:
                nk_d[d] = nk
                break
        else:
            raise ValueError("impulse response decays too slowly")
    return nk_d


def _toeplitz_pair(g, d, j):
    """W[t', t] = g[j*BLK + t - t', d] (zero where k<0), float64."""
    w = np.zeros((BLK, BLK), dtype=np.float64)
    for tp in range(BLK):
        ks = j * BLK - tp
        lo_t = max(0, -ks)
        w[tp, lo_t:] = g[ks + lo_t : ks + BLK, d]
    return w


def _split_waits(nc, limit=1):
    """Walrus in this container rejects instructions carrying more than a
    couple of sync waits.  Move excess waits onto same-engine NOPs placed
    immediately before the offending instruction (program order on the
    engine queue preserves the semantics)."""
    import bass_rust
    import concourse.mybir as mybir

    n_split = 0
    for bb_name, bassbb in list(nc.bb_map.items()):
        bb = bassbb.bb
        insts = list(bb.instructions)
        out = []
        changed = False
        for inst in insts:
            si = inst.sync_info
            if si is not None and len(si.on_wait) > limit:
                waits = list(si.on_wait)
                keep = waits[:limit]
                rest = waits[limit:]
                while rest:
                    chunk, rest = rest[:limit], rest[limit:]
                    nop = bass_rust.InstNoOp(
                        name=f"waitsplit-{n_split}", engine=inst.engine
                    )
                    n_split += 1
                    nop.sync_info = mybir.SyncInfo(on_wait=chunk, on_update=[])
                    nc.register_instruction(nop)
                    out.append(nop)
                inst.sync_info = mybir.SyncInfo(
                    on_wait=keep, on_update=list(si.on_update)
                )
                changed = True
            out.append(inst)
        if changed:
            bb.instructions = out
    return n_split


def _tile_context_cls():
    from concourse.tile import TileContext
    from concourse.vector_clock import ScopedClock, VectorClock

    class TileContextFix(TileContext):
        # This walrus build rejects >2 sync waits on one CTRL instruction
        # ("Too many sync wait commands"), which the stock final drain hits.
        # Split the final-drain waits one-per-NOP on SP; the drain then
        # needs none (program order on SP covers it).
        def _drain_and_barrier(self, tick_clock, wait_clock):
            ticks = list(tick_clock.global_clock)
            for proc, tick in enumerate(ticks):
                if tick <= 0:
                    continue
                nop = self.nc.sync.nop(nofuse=True, hint="drain_wait_split")
                sub = VectorClock(
                    [tick if i == proc else 0 for i in range(len(ticks))]
                )
                wait_clock.add_sem_waits(nop.ins, ScopedClock({None: sub}))
            self.nc.sync.drain()
            self.nc.all_engine_barrier()
            assert self.sems is not None
            popped = self.nc._tile_sem_poison_stack.pop()
            assert popped is self._sem_poison
            self.nc.clear_and_free_semaphores(list(self.sems.allocated().values()))
            self.nc.all_engine_barrier()

    return TileContextFix


def _group_pairs(nk_key):
    """Per group: list of (d, j) pairs, d within the group."""
    groups = []
    for g in range(NG):
        pairs = []
        for d in range(g * DG, (g + 1) * DG):
            for j in range(nk_key[d]):
                pairs.append((d, j))
        groups.append(pairs)
    return groups


def _build_bass(nk_key):
    import concourse.bass as bass
    import concourse.mybir as mybir

    TileContextFix = _tile_context_cls()
    f16, f32 = mybir.dt.float16, mybir.dt.float32
    nk_d = list(nk_key)
    groups = _group_pairs(nk_key)
    goff = [0]
    for g in range(NG):
        goff.append(goff[-1] + len(groups[g]))
    npairs = goff[-1]

    nc = bass.Bass()
    # input: SBUF-image [chunk][128][CS*TB*D] fp16, contiguous, (d, s, tb) cols
    e_p = nc.declare_dram_parameter("e", [NCHUNK, BLK, CS * TB * D], f16, isOutput=False)
    wh_p = nc.declare_dram_parameter("wh", [BLK, npairs * BLK], f16, isOutput=False)
    # output: SBUF-image [chunk][group][128][DG*CS*TB] fp16, contiguous
    xout = nc.declare_dram_parameter("x", [NCHUNK, NG, BLK, DG * CS * TB], f16, isOutput=True)

    with TileContextFix(nc) as tc:
        with (
            tc.tile_pool(name="wpool", bufs=1) as wpool,
            tc.tile_pool(name="epool", bufs=2) as epool,
            tc.tile_pool(name="opool", bufs=4) as opool,
            tc.tile_pool(name="pspool", bufs=8, space="PSUM") as pspool,
        ):
            # all Toeplitz weights resident in SBUF for the whole kernel
            wt = wpool.tile([BLK, npairs * BLK], f16, name="wt")
            nc.sync.dma_start(out=wt[:], in_=wh_p[:])
            pair_idx = {}
            for g in range(NG):
                for i, p_ in enumerate(groups[g]):
                    pair_idx[p_] = goff[g] + i

            for chunk in range(NCHUNK):
                et = epool.tile([BLK, CS * TB * D], f16, name="et")
                nc.sync.dma_start(out=et[:], in_=e_p[chunk])
                # et view: [p][d][s][tb] — channel cols contiguous
                e4 = et.rearrange("p (d s c) -> p d s c", d=D, s=CS)
                for g in range(NG):
                    ot = opool.tile([BLK, DG * NCOL], f16, name="ot")
                    for dl in range(DG):
                        d = g * DG + dl
                        ps = pspool.tile([BLK, NCOL], f32, name="ps")
                        ps3 = ps.rearrange("p (s c) -> p s c", s=CS)
                        nkd = nk_d[d]
                        for j in range(nkd):
                            wi = pair_idx[(d, j)]
                            nc.tensor.matmul(
                                ps3[:, :, j:],
                                wt[:, wi * BLK : (wi + 1) * BLK],
                                e4[:, d, :, 0 : TB - j],
                                start=(j == 0),
                                stop=(j == nkd - 1),
                            )
                        dst = ot[:, dl * NCOL : (dl + 1) * NCOL]
                        if dl % 2 == 0:
                            nc.scalar.copy(out=dst, in_=ps[:])
                        else:
                            nc.vector.tensor_copy(out=dst, in_=ps[:])
                    nc.sync.dma_start(out=xout[chunk, g], in_=ot[:])
    _split_waits(nc)
    return nc, groups


def _marshal_inputs(u16, nk_key):
    """Host-side SBUF-image marshaling."""
    # [core, chunk, s, tb, p, d] -> [core, chunk, p, (d, s, tb)]
    a = u16.reshape(NCORES, NCHUNK, CS, TB, BLK, D)
    a = np.ascontiguousarray(a.transpose(0, 1, 4, 5, 2, 3))
    e_dev = a.reshape(NCORES, NCHUNK, BLK, CS * TB * D)

    groups = _group_pairs(nk_key)
    pairs = [p for g in groups for p in g]
    g64 = _MARSHAL_G[0]
    wh = np.empty((BLK, len(pairs), BLK), dtype=np.float16)
    for i, (d, j) in enumerate(pairs):
        wh[:, i, :] = _toeplitz_pair(g64, d, j).astype(np.float16)
    return e_dev, {"wh": np.ascontiguousarray(wh.reshape(BLK, -1))}


def _unmarshal_output(res_list):
    x = np.empty((N, T, D), dtype=np.float32)
    for c in range(NCORES):
        arr = res_list[c]["x"]  # [chunk, g, p, DG*CS*TB] fp16
        a = arr.reshape(NCHUNK, NG, BLK, DG, CS, TB)
        # -> [chunk, s, tb, p, g, dl]
        a = a.transpose(0, 4, 5, 2, 1, 3)
        x[c * SEQ_PER_CORE : (c + 1) * SEQ_PER_CORE] = a.reshape(
            SEQ_PER_CORE, T, D
        ).astype(np.float32)
    return x


def kernel(eps, phi, theta, mu, x0):
    global LAST_EXEC_NS
    eps = np.asarray(eps, dtype=np.float32)
    phi = np.asarray(phi, dtype=np.float32)
    theta = np.asarray(theta, dtype=np.float32)
    mu = np.asarray(mu, dtype=np.float32)
    x0 = np.asarray(x0, dtype=np.float32)

    g = _impulse_response(phi, theta)
    _MARSHAL_G[0] = g
    c = _mu_offset(theta, mu)
    nk_d = _pick_nk(g, np.abs(c).max(axis=0))
    nk_key = tuple(int(v) for v in nk_d)

    u16 = (eps.astype(np.float64) + c[None, :, :]).astype(np.float16)
    e_dev, wmaps = _marshal_inputs(u16, nk_key)

    if nk_key not in _CACHE:
        _CACHE[nk_key] = _build_bass(nk_key)
    nc, _groups = _CACHE[nk_key]

    from concourse.bass_utils import run_bass_kernel_spmd

    core_ids = list(range(NCORES))
    in_maps = [
        {"e": np.ascontiguousarray(e_dev[cid]), **wmaps} for cid in core_ids
    ]
    trace = bool(int(os.environ.get("ARMA_TRACE", "0")))
    res = run_bass_kernel_spmd(nc, in_maps, core_ids, trace=trace)
    LAST_EXEC_NS = res.exec_time_ns

    x = _unmarshal_output(res.results)

    if np.any(x0):
        h0 = np.zeros((T, D), dtype=np.float64)
        phi64 = phi.astype(np.float64)
        hist = [np.zeros(D)] * 3 + [np.ones(D)]
        for t in range(T):
            val = (
                phi64[:, 0] * hist[3]
                + phi64[:, 1] * hist[2]
                + phi64[:, 2] * hist[1]
                + phi64[:, 3] * hist[0]
            )
            h0[t] = val
            hist = hist[1:] + [val]
        x = x + (x0[:, None, :] * h0[None, :, :]).astype(np.float32)
    return x


# revision 6
# speedup vs baseline: 2.5643x; 1.0394x over previous
"""ARMA(4,4) generator as a truncated-impulse-response convolution on TRN2.

Math: the reference recurrence
    x_t = mu + sum_i phi_i x_{t-i} + e_t + sum_j theta_j e_{t-j}
is a linear time-invariant filter applied to u_t = e_t + c_t (where c solves
c_t + sum_j theta_j c_{t-j} = mu, making the mu term exact), independently per
(sequence, channel):
    x[n, t, d] = sum_k g_d[k] * u[n, t-k, d]   (+ response to x0, zero here)
with g_d the ARMA impulse response (g[0] = 1).  g decays geometrically
(stationary filter); each channel's 128-tap block count NK_d is chosen at
runtime from the actual phi/theta so the truncation error stays below 3e-5.

Device kernel: per channel d, the causal convolution over a 128-step time
block is a lower-triangular block-Toeplitz matmul — time on SBUF partitions,
(sequence, block) pairs as matmul columns.  No serial recurrence remains.

Performance: the kernel is DMA-bound (cost model: all transfers serialize at
360 GB/s), so everything is fp16 end-to-end — inputs, Toeplitz weights and
the output round-trip HBM at 2 bytes/elem, and the weights are loaded once
(not per chunk).  fp32 PSUM accumulation keeps the conv exact; the only
losses are the three fp16 quantizations (~3e-4 rel err vs the fp32
reference, gate is 2e-2).  Traffic per core: 16.75 MB in + 4 MB weights +
16.75 MB out = 37.5 MB -> ~105 us at the 360 GB/s cost-model bandwidth,
vs 99 MB / ~275 us for the fp32+hi/lo variant.

Sharding: pure data parallelism — 32 of the 256 sequences per NeuronCore.
"""

import os
import numpy as np

N, T, D, P, Q = 256, 4096, 64, 4, 4
NCORES = 8
SEQ_PER_CORE = N // NCORES          # 32
BLK = 128                           # time block = SBUF partition count
TB = T // BLK                       # 32 time blocks per sequence
KMAX = 1280                         # host impulse-response horizon
NK_CAP = 8

CS = 8                      # sequences per chunk
NCHUNK = SEQ_PER_CORE // CS  # 4
NG = 4                      # channel groups
DG = D // NG                # 16 channels per group
NCOL = CS * TB              # 256 (s, tb) columns per channel

_CACHE = {}
LAST_EXEC_NS = None
_MARSHAL_G = [None]


def _impulse_response(phi, theta):
    """g[k, d] in float64 for k = 0..KMAX-1."""
    g = np.zeros((KMAX, D), dtype=np.float64)
    g[0] = 1.0
    phi64 = phi.astype(np.float64)
    th64 = theta.astype(np.float64)
    for k in range(1, KMAX):
        acc = np.zeros(D, dtype=np.float64)
        if k <= Q:
            acc += th64[:, k - 1]
        for i in range(1, P + 1):
            if k - i >= 0:
                acc += phi64[:, i - 1] * g[k - i]
        g[k] = acc
    return g


def _mu_offset(theta, mu):
    """c[t, d] with c_t + sum_j theta_j c_{t-j} = mu for all t >= 0.

    Adding c to eps makes the single ARMA filter g reproduce the mu term
    exactly (mu must not pass through the MA part, so a constant offset
    would be wrong during the first Q steps)."""
    th64 = theta.astype(np.float64)
    mu64 = mu.astype(np.float64)
    c = np.zeros((T, D), dtype=np.float64)
    for t in range(T):
        acc = mu64.copy()
        for j in range(1, Q + 1):
            if t - j >= 0:
                acc -= th64[:, j - 1] * c[t - j]
        c[t] = acc
    if np.abs(c).max() > 100.0 * max(np.abs(mu64).max(), 1.0):
        raise ValueError("MA polynomial near non-invertible; c_t diverges")
    return c


def _pick_nk(g, dc_scale):
    """Per-channel tap-block count: coherent DC bias + 6-sigma random tail."""
    mu64 = np.abs(np.asarray(dc_scale, dtype=np.float64))
    nk_d = np.zeros(D, dtype=int)
    for d in range(D):
        for nk in range(2, NK_CAP + 1):
            # block j covers taps j*BLK + t - t'; at output position t=0 the
            # guaranteed coverage ends at (nk-1)*BLK.
            tail = g[(nk - 1) * BLK + 1 :, d]
            bias = abs(tail.sum()) * mu64[d]
            sigma = np.sqrt((tail**2).sum())
            if bias + 6 * sigma < 3e-5:
                nk_d[d] = nk
                break
        else:
            raise ValueError("impulse response decays too slowly")
    return nk_d


def _pick_kd(g, dc_scale, nk_d):
    """Per-channel tap horizon K_d: taps with lag > K_d are negligible under
    the same bias+6-sigma criterion as _pick_nk.  Used to trim all-zero
    trailing columns off the j>=1 Toeplitz blocks."""
    mu64 = np.abs(np.asarray(dc_scale, dtype=np.float64))
    kd = np.zeros(D, dtype=int)
    for d in range(D):
        hi = (nk_d[d] - 1) * BLK  # _pick_nk guarantees this horizon works
        for K in range(1, hi + 1):
            tail = g[K + 1 :, d]
            bias = abs(tail.sum()) * mu64[d]
            sigma = np.sqrt((tail**2).sum())
            if bias + 6 * sigma < 3e-5:
                kd[d] = K
                break
        else:
            kd[d] = hi
    return kd


def _pair_ncol(nk_key, kd_key):
    """Stored column count for each (d, j) Toeplitz block: column t of W_j is
    all-zero once its minimum lag j*BLK + t - (BLK-1) exceeds K_d."""
    ncol = {}
    for d in range(D):
        for j in range(nk_key[d]):
            if j == 0:
                ncol[(d, j)] = BLK
            else:
                ncol[(d, j)] = min(BLK, max(1, kd_key[d] - j * BLK + BLK))
    return ncol


def _toeplitz_pair(g, d, j):
    """W[t', t] = g[j*BLK + t - t', d] (zero where k<0), float64."""
    w = np.zeros((BLK, BLK), dtype=np.float64)
    for tp in range(BLK):
        ks = j * BLK - tp
        lo_t = max(0, -ks)
        w[tp, lo_t:] = g[ks + lo_t : ks + BLK, d]
    return w


def _split_waits(nc, limit=1):
    """Walrus in this container rejects instructions carrying more than a
    couple of sync waits.  Move excess waits onto same-engine NOPs placed
    immediately before the offending instruction (program order on the
    engine queue preserves the semantics)."""
    import bass_rust
    import concourse.mybir as mybir

    n_split = 0
    for bb_name, bassbb in list(nc.bb_map.items()):
        bb = bassbb.bb
        insts = list(bb.instructions)
        out = []
        changed = False
        for inst in insts:
            si = inst.sync_info
            if si is not None and len(si.on_wait) > limit:
                waits = list(si.on_wait)
                keep = waits[:limit]
                rest = waits[limit:]
                while rest:
                    chunk, rest = rest[:limit], rest[limit:]
                    nop = bass_rust.InstNoOp(
                        name=f"waitsplit-{n_split}", engine=inst.engine
                    )
                    n_split += 1
                    nop.sync_info = mybir.SyncInfo(on_wait=chunk, on_update=[])
                    nc.register_instruction(nop)
                    out.append(nop)
                inst.sync_info = mybir.SyncInfo(
                    on_wait=keep, on_update=list(si.on_update)
                )
                changed = True
            out.append(inst)
        if changed:
            bb.instructions = out
    return n_split


def _tile_context_cls():
    from concourse.tile import TileContext
    from concourse.vector_clock import ScopedClock, VectorClock

    class TileContextFix(TileContext):
        # This walrus build rejects >2 sync waits on one CTRL instruction
        # ("Too many sync wait commands"), which the stock final drain hits.
        # Split the final-drain waits one-per-NOP on SP; the drain then
        # needs none (program order on SP covers it).
        def _drain_and_barrier(self, tick_clock, wait_clock):
            ticks = list(tick_clock.global_clock)
            for proc, tick in enumerate(ticks):
                if tick <= 0:
                    continue
                nop = self.nc.sync.nop(nofuse=True, hint="drain_wait_split")
                sub = VectorClock(
                    [tick if i == proc else 0 for i in range(len(ticks))]
                )
                wait_clock.add_sem_waits(nop.ins, ScopedClock({None: sub}))
            self.nc.sync.drain()
            self.nc.all_engine_barrier()
            assert self.sems is not None
            popped = self.nc._tile_sem_poison_stack.pop()
            assert popped is self._sem_poison
            self.nc.clear_and_free_semaphores(list(self.sems.allocated().values()))
            self.nc.all_engine_barrier()

    return TileContextFix


def _group_pairs(nk_key):
    """Per group: list of (d, j) pairs, d within the group."""
    groups = []
    for g in range(NG):
        pairs = []
        for d in range(g * DG, (g + 1) * DG):
            for j in range(nk_key[d]):
                pairs.append((d, j))
        groups.append(pairs)
    return groups


def _build_bass(nk_key, kd_key):
    import concourse.bass as bass
    import concourse.mybir as mybir

    TileContextFix = _tile_context_cls()
    f16, f32 = mybir.dt.float16, mybir.dt.float32
    nk_d = list(nk_key)
    groups = _group_pairs(nk_key)
    ncol = _pair_ncol(nk_key, kd_key)
    pairs = [p for gg in groups for p in gg]
    # column offset of each stored (column-trimmed) Toeplitz block
    poff = {}
    off = 0
    for p_ in pairs:
        poff[p_] = off
        off += ncol[p_]
    wcols = off

    nc = bass.Bass()
    # input: SBUF-image [chunk][128][CS*TB*D] fp16, contiguous, (d, s, tb) cols
    e_p = nc.declare_dram_parameter("e", [NCHUNK, BLK, CS * TB * D], f16, isOutput=False)
    wh_p = nc.declare_dram_parameter("wh", [BLK, wcols], f16, isOutput=False)
    # output: SBUF-image [chunk][group][128][DG*CS*TB] fp16, contiguous
    xout = nc.declare_dram_parameter("x", [NCHUNK, NG, BLK, DG * CS * TB], f16, isOutput=True)

    with TileContextFix(nc) as tc:
        with (
            tc.tile_pool(name="wpool", bufs=1) as wpool,
            tc.tile_pool(name="epool", bufs=4) as epool,
            tc.tile_pool(name="opool", bufs=4) as opool,
            tc.tile_pool(name="pspool", bufs=8, space="PSUM") as pspool,
        ):
            # all Toeplitz weights resident in SBUF for the whole kernel
            wt = wpool.tile([BLK, wcols], f16, name="wt")
            nc.sync.dma_start(out=wt[:], in_=wh_p[:])

            for chunk in range(NCHUNK):
                et = epool.tile([BLK, CS * TB * D], f16, name="et")
                nc.sync.dma_start(out=et[:], in_=e_p[chunk])
                # et view: [p][d][s][tb] — channel cols contiguous
                e4 = et.rearrange("p (d s c) -> p d s c", d=D, s=CS)
                for g in range(NG):
                    ot = opool.tile([BLK, DG * NCOL], f16, name="ot")
                    for dl in range(DG):
                        d = g * DG + dl
                        ps = pspool.tile([BLK, NCOL], f32, name="ps")
                        ps3 = ps.rearrange("p (s c) -> p s c", s=CS)
                        nkd = nk_d[d]
                        for j in range(nkd):
                            o, w = poff[(d, j)], ncol[(d, j)]
                            # j>=1 blocks only touch output rows t < w (the
                            # trailing columns of the full 128-wide block are
                            # all-zero taps beyond the channel's horizon K_d)
                            nc.tensor.matmul(
                                ps3[0:w, :, j:],
                                wt[:, o : o + w],
                                e4[:, d, :, 0 : TB - j],
                                start=(j == 0),
                                stop=(j == nkd - 1),
                            )
                        dst = ot[:, dl * NCOL : (dl + 1) * NCOL]
                        if dl % 2 == 0:
                            nc.scalar.copy(out=dst, in_=ps[:])
                        else:
                            nc.vector.tensor_copy(out=dst, in_=ps[:])
                    nc.sync.dma_start(out=xout[chunk, g], in_=ot[:])
    _split_waits(nc)
    return nc, (pairs, poff, ncol, wcols)


def _marshal_inputs(u16, pairs, poff, ncol, wcols):
    """Host-side SBUF-image marshaling."""
    # [core, chunk, s, tb, p, d] -> [core, chunk, p, (d, s, tb)]
    a = u16.reshape(NCORES, NCHUNK, CS, TB, BLK, D)
    a = np.ascontiguousarray(a.transpose(0, 1, 4, 5, 2, 3))
    e_dev = a.reshape(NCORES, NCHUNK, BLK, CS * TB * D)

    g64 = _MARSHAL_G[0]
    wh = np.zeros((BLK, wcols), dtype=np.float16)
    for p_ in pairs:
        d, j = p_
        o, w = poff[p_], ncol[p_]
        wh[:, o : o + w] = _toeplitz_pair(g64, d, j)[:, :w].astype(np.float16)
    return e_dev, {"wh": np.ascontiguousarray(wh)}


def _unmarshal_output(res_list):
    x = np.empty((N, T, D), dtype=np.float32)
    for c in range(NCORES):
        arr = res_list[c]["x"]  # [chunk, g, p, DG*CS*TB] fp16
        a = arr.reshape(NCHUNK, NG, BLK, DG, CS, TB)
        # -> [chunk, s, tb, p, g, dl]
        a = a.transpose(0, 4, 5, 2, 1, 3)
        x[c * SEQ_PER_CORE : (c + 1) * SEQ_PER_CORE] = a.reshape(
            SEQ_PER_CORE, T, D
        ).astype(np.float32)
    return x


def kernel(eps, phi, theta, mu, x0):
    global LAST_EXEC_NS
    eps = np.asarray(eps, dtype=np.float32)
    phi = np.asarray(phi, dtype=np.float32)
    theta = np.asarray(theta, dtype=np.float32)
    mu = np.asarray(mu, dtype=np.float32)
    x0 = np.asarray(x0, dtype=np.float32)

    g = _impulse_response(phi, theta)
    _MARSHAL_G[0] = g
    c = _mu_offset(theta, mu)
    dc = np.abs(c).max(axis=0)
    nk_d = _pick_nk(g, dc)
    nk_key = tuple(int(v) for v in nk_d)
    kd_key = tuple(int(v) for v in _pick_kd(g, dc, nk_d))

    u16 = (eps.astype(np.float64) + c[None, :, :]).astype(np.float16)

    cache_key = (nk_key, kd_key)
    if cache_key not in _CACHE:
        _CACHE[cache_key] = _build_bass(nk_key, kd_key)
    nc, (pairs, poff, ncol, wcols) = _CACHE[cache_key]
    e_dev, wmaps = _marshal_inputs(u16, pairs, poff, ncol, wcols)

    from concourse.bass_utils import run_bass_kernel_spmd

    core_ids = list(range(NCORES))
    in_maps = [
        {"e": np.ascontiguousarray(e_dev[cid]), **wmaps} for cid in core_ids
    ]
    trace = bool(int(os.environ.get("ARMA_TRACE", "0")))
    res = run_bass_kernel_spmd(nc, in_maps, core_ids, trace=trace)
    LAST_EXEC_NS = res.exec_time_ns

    x = _unmarshal_output(res.results)

    if np.any(x0):
        h0 = np.zeros((T, D), dtype=np.float64)
        phi64 = phi.astype(np.float64)
        hist = [np.zeros(D)] * 3 + [np.ones(D)]
        for t in range(T):
            val = (
                phi64[:, 0] * hist[3]
                + phi64[:, 1] * hist[2]
                + phi64[:, 2] * hist[1]
                + phi64[:, 3] * hist[0]
            )
            h0[t] = val
            hist = hist[1:] + [val]
        x = x + (x0[:, None, :] * h0[None, :, :]).astype(np.float32)
    return x
